# revision 39
# baseline (speedup 1.0000x reference)
"""Axial attention (attention over T axis) on 8 TRN2 NeuronCores.

Problem: q,k,v of shape (2, 8, 128, 16, 16, 128) f32; attention over axis 2
(T=128) with head dim C=128 -> 4096 independent (128x128x128) attention
problems. Data-parallel: 512 problems per core, no inter-core communication.

Device kernel (bf16 matmuls, f32 PSUM accumulation), per half-block of
HB=4 problems:
  sT_j = kT_j.T @ qT_j        (PE; scoresT, Lk x Lq, into shared PSUM tile)
  pT   = exp(sT / sqrt(C))    (one batched ACT op; bf16 SBUF; no max-
                               subtraction needed: |scores/sqrt(C)| < ~6 so
                               fp32/bf16 exp cannot overflow)
  U_j  = pT_j.T @ v_j         (PE; unnormalized out, shared PSUM tile)
  d_j  = pT_j.T @ ones        (PE; N=1 matmul -> exact f32 softmax dents)
  pP_j = transpose(pT_j)      (PE transpose via identity, shared PSUM tile)
  r    = 1/d                  (one batched DVE reciprocal per half-block)
  out  = U * r                (one batched DVE mult, step-0 broadcast of r)
  attn = pP * r               (batched DVE mult for odd halves, per-problem
                               ACT scalar.mul for even halves: load balance)

I/O is bf16 (rel err ~3.5e-3 vs the f32 reference, gate is 2e-2): per core
48MB in + 32MB out, which makes the kernel DMA-bound at ~306GB/s/core.
One input slab DMA ([kt|qt|v]) and one output slab DMA ([attn|out]) per
64-problem block; block 0's input and the last block's output are split so
the pipeline ramp head/tail overlap compute.

Host does layout only: pre-transpose q/k to C-major, pack blocks of BLK
problems into contiguous SBUF-ready slabs, f32->bf16 casts, and the inverse
unshuffle on outputs.
"""

import contextlib
import math

import numpy as np
import ml_dtypes

import concourse.bass as bass
import concourse.bacc as bacc
import concourse.mybir as mybir
import concourse.tile as tile
from concourse.bass_utils import run_bass_kernel_spmd
from concourse.masks import make_identity

B, H, T, HH, WW, C = 2, 8, 128, 16, 16, 128
L = T                      # attention length
N = B * H * HH * WW        # 4096 independent problems
NCORES = 8
NLOC = N // NCORES         # 512 problems per core
BLK = 64                   # problems per DMA block
HB = 4                     # problems per PSUM half-block
NBLK = NLOC // BLK         # blocks per core
SCALE = 1.0 / math.sqrt(C)

BF16 = mybir.dt.bfloat16
F32 = mybir.dt.float32
NP_BF16 = ml_dtypes.bfloat16

LAST_EXEC_NS = None


def _build_nc(nblk_run=NBLK, reps=1):
    nc = bacc.Bacc()

    # single input slab per block: [kt | qt | v] concatenated on the free axis;
    # single output slab per block: [attn | out]
    in_d = nc.declare_dram_parameter("inp", [NBLK, 128, 3 * BLK * L], BF16, isOutput=False)
    out_d = nc.declare_dram_parameter("outp", [NBLK, 128, 2 * BLK * L], BF16, isOutput=True)

    with tile.TileContext(nc) as tc:
        with (
            tc.tile_pool(name="const", bufs=1) as cpool,
            tc.tile_pool(name="ins", bufs=2) as inpool,
            tc.tile_pool(name="pt", bufs=6) as ptpool,
            tc.tile_pool(name="outs", bufs=2) as outpool,
            tc.tile_pool(name="small", bufs=8) as smallpool,
            tc.tile_pool(name="ps_s", bufs=2, space=bass.MemorySpace.PSUM) as ps_s,
            tc.tile_pool(name="ps_u", bufs=2, space=bass.MemorySpace.PSUM) as ps_u,
            tc.tile_pool(name="ps_d", bufs=2, space=bass.MemorySpace.PSUM) as ps_d,
            tc.tile_pool(name="ps_p", bufs=2, space=bass.MemorySpace.PSUM) as ps_p,
        ):
            ident = cpool.tile([128, 128], BF16)
            make_identity(nc, ident[:])
            ones = cpool.tile([128, 1], BF16)
            nc.gpsimd.memset(ones[:], 1.0)

            rep_ctx = tc.For_i(0, reps, 1) if reps > 1 else contextlib.nullcontext()
            with rep_ctx:
                for ib in range(nblk_run):
                    in_t = inpool.tile([128, 3 * BLK * L], BF16, tag="in")
                    # block 0: split by section (kt/qt/v) so the first matmuls
                    # start before v has landed (pipeline ramp)
                    nsp = 3 if ib == 0 else 1
                    wi = 3 * BLK * L // nsp
                    for s in range(nsp):
                        nc.sync.dma_start(
                            in_t[:, s * wi:(s + 1) * wi],
                            in_d[ib][:, s * wi:(s + 1) * wi])
                    kt_t = in_t[:, 0:BLK * L]
                    qt_t = in_t[:, BLK * L:2 * BLK * L]
                    v_t = in_t[:, 2 * BLK * L:3 * BLK * L]

                    o_t = outpool.tile([128, 2 * BLK * L], BF16, tag="o")
                    attn_t = o_t[:, 0:BLK * L]
                    out_t = o_t[:, BLK * L:2 * BLK * L]

                    for h in range(BLK // HB):
                        sT = ps_s.tile([128, HB * L], F32, tag="sT")
                        for jj in range(HB):
                            j = h * HB + jj
                            nc.tensor.matmul(
                                sT[:, jj * L:(jj + 1) * L],
                                kt_t[:, j * L:(j + 1) * L],
                                qt_t[:, j * L:(j + 1) * L],
                                start=True, stop=True,
                            )
                        pT = ptpool.tile([128, HB * L], BF16, tag="pT")
                        nc.scalar.activation(
                            pT[:], sT[:], mybir.ActivationFunctionType.Exp, scale=SCALE
                        )

                        U = ps_u.tile([128, HB * C], F32, tag="U")
                        d = ps_d.tile([128, HB], F32, tag="d")
                        pP_t = ps_p.tile([128, HB * L], BF16, tag="pP")
                        for jj in range(HB):
                            j = h * HB + jj
                            nc.tensor.matmul(
                                U[:, jj * C:(jj + 1) * C],
                                pT[:, jj * L:(jj + 1) * L],
                                v_t[:, j * C:(j + 1) * C],
                                start=True, stop=True,
                            )
                            nc.tensor.matmul(
                                d[:, jj:jj + 1],
                                pT[:, jj * L:(jj + 1) * L],
                                ones[:],
                                start=True, stop=True,
                            )
                            nc.tensor.transpose(
                                pP_t[:, jj * L:(jj + 1) * L],
                                pT[:, jj * L:(jj + 1) * L],
                                ident[:],
                            )
                        pP = pP_t[:].rearrange("p (j l) -> p j l", j=HB)
                        r = smallpool.tile([128, HB], F32, tag="r")
                        nc.vector.reciprocal(r[:], d[:])

                        # out = U * r  (batched, r broadcast along C)
                        nc.vector.tensor_mul(
                            out_t[:, h * HB * C:(h + 1) * HB * C].rearrange(
                                "p (j c) -> p j c", j=HB),
                            U[:].rearrange("p (j c) -> p j c", j=HB),
                            r[:].to_broadcast([128, HB, C]),
                        )
                        # attn = pP * r: batched DVE for odd halves, per-problem
                        # ACT for even halves (engine load balance)
                        if h % 2 == 1:
                            nc.vector.tensor_mul(
                                attn_t[:, h * HB * L:(h + 1) * HB * L].rearrange(
                                    "p (j l) -> p j l", j=HB),
                                pP,
                                r[:].to_broadcast([128, HB, L]),
                            )
                        else:
                            for jj in range(HB):
                                j = h * HB + jj
                                nc.scalar.mul(
                                    attn_t[:, j * L:(j + 1) * L],
                                    pP_t[:, jj * L:(jj + 1) * L],
                                    r[:, jj:jj + 1],
                                )

                    # last block: chunked output DMA so finished halves depart
                    # while the tail halves still compute
                    nsp = 4 if ib == nblk_run - 1 else 1
                    wo = 2 * BLK * L // nsp
                    for s in range(nsp):
                        nc.sync.dma_start(
                            out_d[ib][:, s * wo:(s + 1) * wo],
                            o_t[:, s * wo:(s + 1) * wo])
    nc.finalize()
    return nc


_NC_CACHE = None


def _get_nc():
    global _NC_CACHE
    if _NC_CACHE is None:
        _NC_CACHE = _build_nc()
    return _NC_CACHE


def prep_in_maps(q, k, v):
    # flatten exactly like the reference: moveaxis(2, -2) then (-1, L, C)
    qf = np.moveaxis(q, 2, 4).reshape(N, L, C)
    kf = np.moveaxis(k, 2, 4).reshape(N, L, C)
    vf = np.moveaxis(v, 2, 4).reshape(N, L, C)

    in_maps = []
    for c in range(NCORES):
        sl = slice(c * NLOC, (c + 1) * NLOC)
        # qT/kT: [ib, c, j, l] <- [ib*BLK+j, l, c]  (C on partitions)
        qs = qf[sl].reshape(NBLK, BLK, L, C).transpose(0, 3, 1, 2)
        ks = kf[sl].reshape(NBLK, BLK, L, C).transpose(0, 3, 1, 2)
        qt = np.ascontiguousarray(qs).astype(NP_BF16).reshape(NBLK, 128, BLK * L)
        kt = np.ascontiguousarray(ks).astype(NP_BF16).reshape(NBLK, 128, BLK * L)
        # v: [ib, lk, j, c]  (Lk on partitions)
        vs = vf[sl].reshape(NBLK, BLK, L, C).transpose(0, 2, 1, 3)
        vv = np.ascontiguousarray(vs).astype(NP_BF16).reshape(NBLK, 128, BLK * C)
        inp = np.concatenate([kt, qt, vv], axis=2)
        in_maps.append({"inp": inp})
    return in_maps


def kernel(q, k, v):
    global LAST_EXEC_NS
    q = np.asarray(q)
    k = np.asarray(k)
    v = np.asarray(v)
    in_maps = prep_in_maps(q, k, v)
    res = run_bass_kernel_spmd(_get_nc(), in_maps, list(range(NCORES)))
    LAST_EXEC_NS = res.exec_time_ns

    attn_parts = []
    out_parts = []
    for c in range(NCORES):
        op = np.asarray(res.results[c]["outp"])
        a = op[:, :, :BLK * L].astype(np.float32)
        o = op[:, :, BLK * L:].astype(np.float32)
        # [ib, lq, j, x] -> [ib, j, lq, x] -> (NLOC, 128, x)
        a = a.reshape(NBLK, 128, BLK, L).transpose(0, 2, 1, 3).reshape(NLOC, L, L)
        o = o.reshape(NBLK, 128, BLK, C).transpose(0, 2, 1, 3).reshape(NLOC, L, C)
        attn_parts.append(a)
        out_parts.append(o)

    attn = np.concatenate(attn_parts, axis=0)
    outf = np.concatenate(out_parts, axis=0)
    out = np.moveaxis(outf.reshape(B, H, HH, WW, T, C), 4, 2)
    return np.ascontiguousarray(out), attn
